# revision 6
# baseline (speedup 1.0000x reference)
"""Trainium2 Bass kernel for CacheShrink MLA attention (8-core SPMD).

Math (matching the reference; dead K/V decompression skipped):
  q = rope(hidden @ Wq) * 1/sqrt(dh)
  c_k, c_v = split(hidden @ Wc)
  per head h (32 heads, GQA onto 4 compressed kv heads):
    S = q_h @ c_k[kv(h)].T  (causal)
    P = exp(S)              (scores are bounded ~[-10, 11], no max needed)
    ctx_h = (P @ c_v[kv(h)]) / rowsum(P)
  out = ctx @ Wo

Sharding: tensor-parallel over heads. Core i owns query heads 4i..4i+3
(all mapping to compressed kv head i//2, so each core computes only its
own 128-dim slice of c_k/c_v from a column slice of Wc). After
attention, bf16 ctx^T shards are AllGather'd (2 MB/rank) and each core
computes a disjoint 512-column block of the output projection, so no
all-reduce is needed. Everything on-chip runs in a transposed layout
(t on the free dim) so every matmul operand is produced in its natural
orientation; the host transposes the final (4096, 2048) result once.

All matmuls are bf16 with f32 PSUM accumulation. The softmax
denominator is computed with a ones-vector matmul over the bf16 probs
(partition-dim reduction on the PE), reciprocal + partition-broadcast,
and folded into ctx before the output projection.
"""

import numpy as np
import ml_dtypes

import concourse.bass as bass
import concourse.mybir as mybir
import concourse.tile as tile
from concourse import bacc
from concourse.bass_utils import run_bass_kernel_spmd

BF16 = mybir.dt.bfloat16
F32 = mybir.dt.float32
PSUM = bass.MemorySpace.PSUM

N_CORES = 8
H_PER_CORE = 4      # query heads per core
DH = 128            # head dim
NKO = 32            # k-tiles over the 4096 ctx rows of Wo (32 heads * 128)
TB = 512            # t-block width (one PSUM bank of f32)


def build_nc(T=2048, DM=4096, repeat=1, collective=True):
    """Build the single-core SPMD program (same for all 8 cores)."""
    NB = T // TB          # 512-wide t blocks
    NT = T // 128         # 128-wide s tiles
    NK = DM // 128        # k-tiles over d_model
    Exp = mybir.ActivationFunctionType.Exp

    nc = bacc.Bacc("TRN2", target_bir_lowering=False, debug=False,
                   num_devices=N_CORES)

    hT = nc.dram_tensor("hT", [DM, T], BF16, kind="ExternalInput")
    wq = nc.dram_tensor("wq", [DM, H_PER_CORE * DH], BF16,
                        kind="ExternalInput")
    wck = nc.dram_tensor("wck", [DM, DH], BF16, kind="ExternalInput")
    wcv = nc.dram_tensor("wcv", [DM, DH], BF16, kind="ExternalInput")
    wo = nc.dram_tensor("wo", [NKO * 128, H_PER_CORE * DH], BF16,
                        kind="ExternalInput")
    cosT = nc.dram_tensor("cosT", [64, T], F32, kind="ExternalInput")
    sinT = nc.dram_tensor("sinT", [64, T], F32, kind="ExternalInput")
    o_t = nc.dram_tensor("o_t", [H_PER_CORE * DH, T], F32,
                         kind="ExternalOutput")

    # internal DRAM for the ctx all-gather
    ctx_loc = nc.dram_tensor("ctx_loc", [H_PER_CORE * DH, T], BF16)
    ctx_all = nc.dram_tensor("ctx_all", [NKO * 128, T], BF16,
                             addr_space="Shared" if collective else "Local")

    with tile.TileContext(nc) as tc:
        with tc.tile_pool(name="persist", bufs=1) as pp:
            # constants
            ident = pp.tile([128, 128], BF16, tag="ident")
            nc.gpsimd.memset(ident[:], 0.0)
            nc.gpsimd.affine_select(
                out=ident[:], in_=ident[:],
                compare_op=mybir.AluOpType.not_equal, fill=1.0,
                base=0, pattern=[[-1, 128]], channel_multiplier=1)
            # mask[s, t] = 1 if s <= t else 0 (applied to the diagonal
            # 128x128 block of probs^T after exp)
            mask = pp.tile([128, 128], BF16, tag="mask")
            nc.gpsimd.memset(mask[:], 1.0)
            nc.gpsimd.affine_select(
                out=mask[:], in_=mask[:],
                compare_op=mybir.AluOpType.is_ge, fill=0.0,
                base=0, pattern=[[1, 128]], channel_multiplier=-1)
            ones = pp.tile([128, 1], BF16, tag="ones")
            nc.gpsimd.memset(ones[:], 1.0)

            # weights + rope tables (resident)
            wq_sb = pp.tile([128, NK, H_PER_CORE * DH], BF16, tag="wq")
            nc.sync.dma_start(wq_sb[:], wq.rearrange("(k p) n -> p k n", p=128))
            wck_sb = pp.tile([128, NK, DH], BF16, tag="wck")
            nc.sync.dma_start(wck_sb[:], wck.rearrange("(k p) n -> p k n", p=128))
            wcv_sb = pp.tile([128, NK, DH], BF16, tag="wcv")
            nc.sync.dma_start(wcv_sb[:], wcv.rearrange("(k p) n -> p k n", p=128))
            wo_sb = pp.tile([128, NKO, H_PER_CORE * DH], BF16, tag="wo")
            nc.sync.dma_start(wo_sb[:], wo.rearrange("(k p) n -> p k n", p=128))
            cos_sb = pp.tile([64, T], F32, tag="cos")
            nc.sync.dma_start(cos_sb[:], cosT[:])
            sin_sb = pp.tile([64, T], F32, tag="sin")
            nc.sync.dma_start(sin_sb[:], sinT[:])

            # per-core activations (persist across phases)
            qrT = [pp.tile([128, T], BF16, tag=f"qrT{h}", name=f"qrT{h}")
                   for h in range(H_PER_CORE)]
            ckT_sb = pp.tile([128, T], BF16, tag="ckT")
            cv_sb = pp.tile([128, T], BF16, tag="cv")  # NT tiles of [s128, d128]

            for _ in range(repeat):
                # ---- Phase AB: q/ck/cv projections (+rope) ----
                with (
                    tc.tile_pool(name="ht", bufs=4) as htp,
                    tc.tile_pool(name="abw", bufs=8) as abw,
                    tc.tile_pool(name="qps", bufs=4, space=PSUM) as qpsp,
                    tc.tile_pool(name="kvps", bufs=2, space=PSUM) as kvpsp,
                    tc.tile_pool(name="trps", bufs=1, space=PSUM) as trpsp,
                ):
                    for b in range(NB):
                        bc = slice(b * TB, (b + 1) * TB)
                        qps = [qpsp.tile([128, TB], F32, tag="q", name=f"qps{b}_{hh}")
                               for hh in range(H_PER_CORE)]
                        ckp = kvpsp.tile([128, TB], F32, tag="ckv")
                        cvp = kvpsp.tile([128, TB], F32, tag="ckv")
                        for k in range(NK):
                            ht = htp.tile([128, TB], BF16, tag="ht")
                            nc.sync.dma_start(ht[:], hT[k * 128:(k + 1) * 128, bc])
                            for h in range(H_PER_CORE):
                                nc.tensor.matmul(
                                    qps[h][:],
                                    wq_sb[:, k, h * DH:(h + 1) * DH], ht[:],
                                    start=(k == 0), stop=(k == NK - 1))
                            nc.tensor.matmul(ckp[:], wck_sb[:, k, :], ht[:],
                                             start=(k == 0), stop=(k == NK - 1))
                            nc.tensor.matmul(cvp[:], wcv_sb[:, k, :], ht[:],
                                             start=(k == 0), stop=(k == NK - 1))
                        # rope drain: qrT = rope(q) (scale folded into tables)
                        for h in range(H_PER_CORE):
                            t1 = abw.tile([64, TB], F32, tag="t1")
                            t2 = abw.tile([64, TB], F32, tag="t2")
                            nc.vector.tensor_mul(t1[:], qps[h][0:64, :], cos_sb[:, bc])
                            nc.vector.tensor_mul(t2[:], qps[h][64:128, :], sin_sb[:, bc])
                            nc.vector.tensor_sub(qrT[h][0:64, bc], t1[:], t2[:])
                            t3 = abw.tile([64, TB], F32, tag="t1")
                            t4 = abw.tile([64, TB], F32, tag="t2")
                            nc.vector.tensor_mul(t3[:], qps[h][64:128, :], cos_sb[:, bc])
                            nc.vector.tensor_mul(t4[:], qps[h][0:64, :], sin_sb[:, bc])
                            nc.vector.tensor_add(qrT[h][64:128, bc], t3[:], t4[:])
                        nc.vector.tensor_copy(ckT_sb[:, bc], ckp[:])
                        cvt = abw.tile([128, TB], BF16, tag="cvt")
                        nc.vector.tensor_copy(cvt[:], cvp[:])
                        for jl in range(TB // 128):
                            j = (TB // 128) * b + jl
                            tp = trpsp.tile([128, 128], BF16, tag="tr")
                            nc.tensor.transpose(
                                tp[:], cvt[:, jl * 128:(jl + 1) * 128], ident[:])
                            nc.vector.tensor_copy(
                                cv_sb[:, j * 128:(j + 1) * 128], tp[:])

                # ---- Phase C: attention (transposed layout) ----
                with (
                    tc.tile_pool(name="cwork", bufs=2) as cw,
                    tc.tile_pool(name="probs", bufs=4) as prp,
                    tc.tile_pool(name="stps", bufs=3, space=PSUM) as stp,
                    tc.tile_pool(name="ctxps", bufs=2, space=PSUM) as ctxp,
                    tc.tile_pool(name="denps", bufs=2, space=PSUM) as denp,
                ):
                    for h in range(H_PER_CORE):
                        for b in range(NB):
                            nj = (TB // 128) * (b + 1)
                            ctxps = ctxp.tile([128, TB], F32, tag="ctx")
                            denps = denp.tile([1, TB], F32, tag="den")
                            probs_t = {}

                            def emit_pv(j, *, _c=ctxps, _d=denps, _p=probs_t,
                                        _b=b, _nj=nj):
                                lo = max(0, 128 * (j - (TB // 128) * _b))
                                pr = _p.pop(j)
                                nc.tensor.matmul(
                                    _c[:, lo:], cv_sb[:, j * 128:(j + 1) * 128],
                                    pr[:, lo:],
                                    start=(j == 0), stop=(j == _nj - 1))
                                nc.tensor.matmul(
                                    _d[:, lo:], ones[:], pr[:, lo:],
                                    start=(j == 0), stop=(j == _nj - 1))

                            for j in range(nj):
                                lo = max(0, 128 * (j - (TB // 128) * b))
                                stps = stp.tile([128, TB], F32, tag="st")
                                nc.tensor.matmul(
                                    stps[:, lo:],
                                    ckT_sb[:, j * 128:(j + 1) * 128],
                                    qrT[h][:, b * TB + lo:(b + 1) * TB],
                                    start=True, stop=True)
                                pr = prp.tile([128, TB], BF16, tag="probs")
                                nc.scalar.activation(pr[:, lo:], stps[:, lo:], Exp)
                                if j >= (TB // 128) * b:
                                    nc.vector.tensor_mul(
                                        pr[:, lo:lo + 128],
                                        pr[:, lo:lo + 128], mask[:])
                                probs_t[j] = pr
                                if j >= 2:
                                    emit_pv(j - 2)
                            emit_pv(nj - 2)
                            emit_pv(nj - 1)

                            rec = cw.tile([1, TB], F32, tag="rec")
                            nc.vector.reciprocal(rec[:], denps[:])
                            bcst = cw.tile([128, TB], F32, tag="bc")
                            nc.gpsimd.partition_broadcast(bcst[:], rec[:])
                            cn = cw.tile([128, TB], BF16, tag="cn")
                            nc.vector.tensor_mul(cn[:], ctxps[:], bcst[:])
                            nc.sync.dma_start(
                                ctx_loc[h * 128:(h + 1) * 128, b * TB:(b + 1) * TB],
                                cn[:])

                # ---- AllGather ctx across the 8 cores ----
                if collective:
                    nc.gpsimd.collective_compute(
                        "AllGather", mybir.AluOpType.bypass,
                        ins=[ctx_loc[:]], outs=[ctx_all[:]],
                        replica_groups=[list(range(N_CORES))])

                # ---- Phase E: output projection (512-col block) ----
                with (
                    tc.tile_pool(name="ctxt", bufs=4) as ctp,
                    tc.tile_pool(name="ost", bufs=2) as ostp,
                    tc.tile_pool(name="ops", bufs=4, space=PSUM) as opsp,
                ):
                    for b in range(NB):
                        bc = slice(b * TB, (b + 1) * TB)
                        oacc = [opsp.tile([128, TB], F32, tag="o", name=f"oacc{b}_{mm}")
                                for mm in range(H_PER_CORE * DH // 128)]
                        for k in range(NKO):
                            ct = ctp.tile([128, TB], BF16, tag="ct")
                            nc.sync.dma_start(
                                ct[:], ctx_all[k * 128:(k + 1) * 128, bc])
                            for m in range(len(oacc)):
                                nc.tensor.matmul(
                                    oacc[m][:],
                                    wo_sb[:, k, m * 128:(m + 1) * 128], ct[:],
                                    start=(k == 0), stop=(k == NKO - 1))
                        for m in range(len(oacc)):
                            ost = ostp.tile([128, TB], F32, tag="ost")
                            nc.vector.tensor_copy(ost[:], oacc[m][:])
                            nc.sync.dma_start(
                                o_t[m * 128:(m + 1) * 128, bc], ost[:])

    nc.compile()
    return nc


_CACHE = {}


def _get_nc(T, DM, repeat=1):
    key = (T, DM, repeat)
    if key not in _CACHE:
        _CACHE[key] = build_nc(T, DM, repeat)
    return _CACHE[key]


def make_inputs(positions, hidden_states, Wq, Wc, Wo, T, DM):
    """Shard + prep the full inputs into 8 per-core input maps."""
    bf = ml_dtypes.bfloat16
    d_latent = Wc.shape[1] // 2
    hT = np.ascontiguousarray(hidden_states.T).astype(bf)

    pos = positions.astype(np.float32)
    inv = (1.0 / (10000.0 ** (np.arange(64, dtype=np.float32) * (2.0 / 128.0))))
    freqs = pos[:, None] * inv[None, :]          # (T, 64) f32
    scale = np.float32(1.0 / np.sqrt(128.0))
    cosT = np.ascontiguousarray((np.cos(freqs) * scale).T)  # (64, T)
    sinT = np.ascontiguousarray((np.sin(freqs) * scale).T)

    in_maps = []
    for i in range(N_CORES):
        kv = i // 2
        in_maps.append({
            "hT": hT,
            "wq": np.ascontiguousarray(
                Wq[:, i * H_PER_CORE * DH:(i + 1) * H_PER_CORE * DH]).astype(bf),
            "wck": np.ascontiguousarray(
                Wc[:, kv * DH:(kv + 1) * DH]).astype(bf),
            "wcv": np.ascontiguousarray(
                Wc[:, d_latent + kv * DH:d_latent + (kv + 1) * DH]).astype(bf),
            "wo": np.ascontiguousarray(
                Wo[:, i * H_PER_CORE * DH:(i + 1) * H_PER_CORE * DH]).astype(bf),
            "cosT": cosT,
            "sinT": sinT,
        })
    return in_maps


def kernel(positions, hidden_states, Wq, Wc, Wuk, Wuv, Wo):
    positions = np.asarray(positions)
    hidden_states = np.asarray(hidden_states, dtype=np.float32)
    Wq = np.asarray(Wq, dtype=np.float32)
    Wc = np.asarray(Wc, dtype=np.float32)
    Wo = np.asarray(Wo, dtype=np.float32)
    T, DM = hidden_states.shape

    nc = _get_nc(T, DM)
    in_maps = make_inputs(positions, hidden_states, Wq, Wc, Wo, T, DM)
    res = run_bass_kernel_spmd(nc, in_maps, list(range(N_CORES))).results
    oT = np.concatenate([res[i]["o_t"] for i in range(N_CORES)], axis=0)
    return np.ascontiguousarray(oT.T)
